# revision 29
# baseline (speedup 1.0000x reference)
"""Causal single-head attention (B=4, T=4096, C=1024, H=64) on 8 TRN2 NeuronCores.

Sharding: 2 cores per batch element; within a batch, the 8 query blocks of 512
rows are split by parity (core s owns blocks {s, s+2, s+4, s+6}), which
balances the causal workload between the two cores.

One SPMD program for all 8 cores:
  - x arrives pre-transposed per batch as [C, T] so the contraction dim C sits
    on SBUF partitions; loaded as [128, 1024] pieces (1 MB-class DMAs), cast
    f32->f16 on DVE.
  - Projections ([Wk|Wv] packed, plus Wq) run for the full batch on both
    cores of a pair in f16 (PE streams 1 col/cycle; fp32 runs 4x slower).
  - Attention computes S^T = K_tile^T @ Q per 128-wide kv tile so that softmax
    needs NO max pass (scores are bounded for this distribution), NO P
    transpose (S^T is already [kv, q]), and the row-sum is folded into the
    P@V matmul via a ones column appended to V. exp runs on ACT with the
    1/sqrt(H) scale fused; causal masking is a multiplicative f16 mask on the
    last 8 kv tiles of each q block (mask data is a per-core input).
  - Attention iterations are interleaved into the projection stream as their
    K/V/Q tiles become available, keeping the PE dense enough for the HAM
    clock to reach 2.4 GHz; per-core q-block offsets come from the
    partition-id register via dynamic access patterns.
  - Per q block the accumulated [O^T; l] PSUM is transposed back on the PE and
    normalized by 1/l on DVE, then DMA'd out.
"""

import numpy as np

import concourse.bacc as bacc
import concourse.bass as bass
import concourse.mybir as mybir
import concourse.tile as tile
from concourse.bass_utils import run_bass_kernel_spmd
from concourse.masks import make_identity

B, T, C, H = 4, 4096, 1024, 64
NCORES = 8
TB = 512                 # q/t block width
NTB = T // TB            # 8 t-blocks for projections
NQB = 4                  # local q blocks per core
NKVT = T // 128          # 32 kv tiles of 128
F32 = mybir.dt.float32
F32R = mybir.dt.float32r
F16 = mybir.dt.float16

_nc = None


def _build():
    nc = bacc.Bacc("TRN2", target_bir_lowering=False, debug=False, num_devices=NCORES)
    xt = nc.dram_tensor("xt", [C, T], F32, kind="ExternalInput").ap()
    wq = nc.dram_tensor("wq", [128, 8 * H], F32, kind="ExternalInput").ap()
    wkv = nc.dram_tensor("wkv", [128, 8 * 2 * H], F32, kind="ExternalInput").ap()
    masks = nc.dram_tensor("masks", [128, 8 * TB], F16, kind="ExternalInput").ap()
    out = nc.dram_tensor("out", [NQB * TB, H], F32, kind="ExternalOutput").ap()

    TSB = 4 * TB  # 2048

    with tile.TileContext(nc) as tc:
        pid = nc.partition_id(engines=[mybir.EngineType.PE])
        s = pid % 2
        with tc.tile_pool(name="persist", bufs=1) as persist, \
             tc.tile_pool(name="x32p", bufs=6) as x32p, \
             tc.tile_pool(name="x16p", bufs=16) as x16p, \
             tc.tile_pool(name="vtp", bufs=2) as vtp, \
             tc.tile_pool(name="otp", bufs=2) as otp, \
             tc.tile_pool(name="obp", bufs=3) as obp, \
             tc.tile_pool(name="rcp", bufs=2) as rcp, \
             tc.tile_pool(name="ptp", bufs=6) as ptp, \
             tc.tile_pool(name="pjp", bufs=2, space="PSUM") as pj_pool, \
             tc.tile_pool(name="pvp", bufs=1, space="PSUM") as pv_pool, \
             tc.tile_pool(name="psp", bufs=2, space="PSUM") as ps_pool, \
             tc.tile_pool(name="pop", bufs=2, space="PSUM") as po_pool:
            ident = persist.tile([128, 128], F32)
            make_identity(nc, ident)
            wq_sb32 = persist.tile([128, 8 * H], F32)
            wkv_sb32 = persist.tile([128, 8 * 2 * H], F32)
            nc.scalar.dma_start(out=wq_sb32, in_=wq)
            nc.scalar.dma_start(out=wkv_sb32, in_=wkv)
            wq_sb = persist.tile([128, 8 * H], F16)
            wkv_sb = persist.tile([128, 8 * 2 * H], F16)
            nc.vector.tensor_copy(wq_sb, wq_sb32)
            nc.vector.tensor_copy(wkv_sb, wkv_sb32)
            masks_sb = persist.tile([128, 8 * TB], F16)
            nc.scalar.dma_start(out=masks_sb, in_=masks)

            QT = persist.tile([64, T], F16)           # Q^T on partitions 0:64
            KT = persist.tile([64, T], F16)           # K^T on partitions 0:64
            V = persist.tile([128, NKVT, H + 1], F16)  # [128, 65] per kv tile
            # col 64 of each kv tile = 1.0 (row-sum column); ACT rounds to f16
            nc.scalar.activation(
                V[:, :, H],
                ident[:, 0:NKVT],
                mybir.ActivationFunctionType.Copy,
                scale=0.0,
                bias=1.0,
            )

            # pre-warm the PE clock while the first x DMAs are in flight
            for w in range(16):
                psum_warm = ps_pool.tile([128, TB], F32, name="psum_warm",
                                         tag="ps")
                nc.tensor.matmul(
                    psum_warm[:, 0:128], ident, ident, start=True, stop=True
                )

            qoffs = [s * TB + i * 2 * TB for i in range(NQB)]
            x16s = [None] * 8

            # ---- attention emission machinery (fused into the proj stream) ----
            st = {"psum_o": None, "next_kp": [0] * NQB, "done": [False] * NQB,
                  "po": [None] * NQB}

            def emit_pair(i, kp):
                nkv = 8 * i + 8
                if kp == 0:
                    st["po"][i] = po_pool.tile([H + 1, TB], F32, name="psum_o",
                                               tag="po")
                psum_o = st["po"][i]
                for h in range(2):
                    k = 2 * kp + h
                    psum_s = ps_pool.tile([128, TB], F32, name="psum_s", tag="ps")
                    nc.tensor.matmul(
                        psum_s,
                        KT[:, k * 128:(k + 1) * 128],
                        QT[:, bass.ds(qoffs[i], TB)],
                        start=True,
                        stop=True,
                    )
                    pt = ptp.tile([128, TB], F16, name="pt", tag="pt")
                    nc.scalar.activation(
                        pt, psum_s, mybir.ActivationFunctionType.Exp, scale=0.125
                    )
                    j = k - (nkv - 8)
                    if j >= 0:
                        nc.vector.tensor_mul(
                            pt, pt, masks_sb[:, j * TB:(j + 1) * TB]
                        )
                    nc.tensor.matmul(
                        psum_o,
                        V[:, k, :],
                        pt,
                        start=(k == 0),
                        stop=(k == nkv - 1),
                    )
                if 2 * kp + 1 == nkv - 1:
                    # epilogue: normalize + store this q block
                    ot = otp.tile([H + 1, TB], F32)
                    nc.vector.tensor_copy(ot, psum_o)
                    for j2 in range(4):
                        psum_t = ps_pool.tile([128, TB], F32, name="psum_t",
                                              tag="ps")
                        nc.tensor.transpose(
                            psum_t[:, 0:H + 1],
                            ot[:, j2 * 128:(j2 + 1) * 128],
                            ident[0:H + 1, 0:H + 1],
                        )
                        rec = rcp.tile([128, 1], F32)
                        nc.vector.reciprocal(rec, psum_t[:, H:H + 1])
                        ob = obp.tile([128, H], F32)
                        nc.vector.tensor_scalar_mul(ob, psum_t[:, 0:H], rec)
                        nc.sync.dma_start(
                            out=out[i * TB + j2 * 128:i * TB + (j2 + 1) * 128, :],
                            in_=ob,
                        )

            def avail_g(i, kp):
                # q block i needs QT global block 2i+s (<= 2i+1); kv pair kp
                # needs proj t-block (2kp+1)//4. Block 3 intentionally waits
                # for the end anyway (its q arrives with the last proj block).
                base = max(2 * i + 1, (2 * kp + 1) // 4)
                if i == 2:
                    base = max(base, 6)
                return base

            def emit_ready(g, budget):
                emitted = 1
                while budget != 0 and emitted:
                    emitted = 0
                    for i in range(NQB):
                        if budget == 0:
                            break
                        kp = st["next_kp"][i]
                        if kp < (8 * i + 8) // 2 and avail_g(i, kp) <= g:
                            emit_pair(i, kp)
                            st["next_kp"][i] = kp + 1
                            emitted = 1
                            budget -= 1

            # ---- fused projection + attention stream ----
            PIECE = 2 * TB  # 1024
            for g in range(NTB):
                if g % 2 == 0:
                    # one [128, 1024] piece per c-chunk covers t-blocks g, g+1;
                    # the very first wave loads/casts in [128, 512] halves so
                    # the first matmuls start sooner
                    p0 = g * TB
                    for c in range(8):
                        x32 = x32p.tile([128, PIECE], F32, name="x32", tag="x32")
                        x16 = x16p.tile([128, PIECE], F16, name="x16", tag="x16")
                        if g == 0:
                            for hh in range(2):
                                hsl = slice(hh * TB, (hh + 1) * TB)
                                nc.sync.dma_start(
                                    out=x32[:, hsl],
                                    in_=xt[c * 128:(c + 1) * 128,
                                           p0 + hh * TB:p0 + (hh + 1) * TB],
                                )
                                nc.vector.tensor_copy(x16[:, hsl], x32[:, hsl])
                        else:
                            nc.sync.dma_start(
                                out=x32,
                                in_=xt[c * 128:(c + 1) * 128, p0:p0 + PIECE],
                            )
                            nc.vector.tensor_copy(x16, x32)
                        x16s[c] = x16
                sl = slice((g % 2) * TB, (g % 2 + 1) * TB)
                psum_vk = pj_pool.tile([128, TB], F32, name="psum_vk", tag="pj")
                for c in range(8):
                    nc.tensor.matmul(
                        psum_vk,
                        wkv_sb[:, c * 128:(c + 1) * 128],
                        x16s[c][:, sl],
                        start=(c == 0),
                        stop=(c == 7),
                    )
                psum_q = pj_pool.tile([64, TB], F32, name="psum_q", tag="pj")
                for c in range(8):
                    nc.tensor.matmul(
                        psum_q,
                        wq_sb[:, c * H:(c + 1) * H],
                        x16s[c][:, sl],
                        start=(c == 0),
                        stop=(c == 7),
                    )
                nc.scalar.copy(QT[:, g * TB:(g + 1) * TB], psum_q)
                nc.scalar.copy(KT[:, g * TB:(g + 1) * TB], psum_vk[0:64, :])
                vt = vtp.tile([128, TB], F32)
                nc.scalar.copy(vt[64:128, :], psum_vk[64:128, :])
                for j in range(4):
                    psum_v = pv_pool.tile([128, H], F32)
                    nc.tensor.transpose(
                        psum_v,
                        vt[64:128, j * 128:(j + 1) * 128],
                        ident[64:128, 64:128],
                    )
                    nc.scalar.copy(V[:, 4 * g + j, 0:H], psum_v)
                # attention filler: a few ready pairs per proj block
                emit_ready(g, 7 if g < NTB - 1 else -1)

    nc.compile()
    return nc


def get_nc():
    global _nc
    if _nc is None:
        _nc = _build()
    return _nc


def make_inputs(x, Wq, Wk, Wv):
    """Build the 8 per-core input maps."""
    x = np.asarray(x, dtype=np.float32)

    def pack_w(wt):
        # [C, M] (= W.T) -> [128, 8*M]: partition p, free c*M+m = wt[c*128+p, m]
        M = wt.shape[1]
        return np.ascontiguousarray(
            wt.reshape(8, 128, M).transpose(1, 0, 2).reshape(128, 8 * M)
        )

    wq_in = pack_w(np.asarray(Wq, np.float32).T)
    wkv_in = pack_w(
        np.concatenate(
            [np.asarray(Wk, np.float32).T, np.asarray(Wv, np.float32).T], axis=1
        )
    )
    p = np.arange(128, dtype=np.int64)[:, None]
    f = np.arange(TB, dtype=np.int64)[None, :]
    masks_by_s = []
    for s in range(2):
        m = np.concatenate(
            [((512 * s + f - 128 * j - p) >= 0).astype(np.float16) for j in range(8)],
            axis=1,
        )
        masks_by_s.append(np.ascontiguousarray(m))
    in_maps = []
    for core in range(NCORES):
        b, s = core // 2, core % 2
        in_maps.append(
            {
                "xt": np.ascontiguousarray(x[b].T),
                "wq": wq_in,
                "wkv": wkv_in,
                "masks": masks_by_s[s],
            }
        )
    return in_maps


def gather_output(results):
    """results: list of per-core {"out": [2048, 64]} -> full [B, T, H]."""
    O = np.empty((B, T, H), np.float32)
    for core in range(NCORES):
        b, s = core // 2, core % 2
        o = results[core]["out"]
        for i in range(NQB):
            g = 2 * i + s
            O[b, g * TB:(g + 1) * TB] = o[i * TB:(i + 1) * TB]
    return O


def kernel(x, Wq, Wk, Wv):
    nc = get_nc()
    in_maps = make_inputs(x, Wq, Wk, Wv)
    res = run_bass_kernel_spmd(nc, in_maps, list(range(NCORES)))
    return gather_output(res.results)


# revision 30
# speedup vs baseline: 1.0531x; 1.0531x over previous
"""Causal single-head attention (B=4, T=4096, C=1024, H=64) on 8 TRN2 NeuronCores.

Sharding: 2 cores per batch element; within a batch, the 8 query blocks of 512
rows are split by parity (core s owns blocks {s, s+2, s+4, s+6}), which
balances the causal workload between the two cores.

One SPMD program for all 8 cores:
  - x arrives pre-transposed per batch as [C, T] so the contraction dim C sits
    on SBUF partitions; loaded as [128, 1024] pieces (1 MB-class DMAs), cast
    f32->f16 on DVE.
  - Projections ([Wk|Wv] packed, plus Wq) run for the full batch on both
    cores of a pair in f16 (PE streams 1 col/cycle; fp32 runs 4x slower).
  - Attention computes S^T = K_tile^T @ Q per 128-wide kv tile so that softmax
    needs NO max pass (scores are bounded for this distribution), NO P
    transpose (S^T is already [kv, q]), and the row-sum is folded into the
    P@V matmul via a ones column appended to V. exp runs on ACT with the
    1/sqrt(H) scale fused; causal masking is a multiplicative f16 mask on the
    last 8 kv tiles of each q block (mask data is a per-core input).
  - Attention iterations are interleaved into the projection stream as their
    K/V/Q tiles become available, keeping the PE dense enough for the HAM
    clock to reach 2.4 GHz; per-core q-block offsets come from the
    partition-id register via dynamic access patterns.
  - Per q block the accumulated [O^T; l] PSUM is transposed back on the PE and
    normalized by 1/l on DVE, then DMA'd out.
"""

import numpy as np

import concourse.bacc as bacc
import concourse.bass as bass
import concourse.mybir as mybir
import concourse.tile as tile
from concourse.bass_utils import run_bass_kernel_spmd
from concourse.masks import make_identity

B, T, C, H = 4, 4096, 1024, 64
NCORES = 8
TB = 512                 # q/t block width
NTB = T // TB            # 8 t-blocks for projections
NQB = 4                  # local q blocks per core
NKVT = T // 128          # 32 kv tiles of 128
F32 = mybir.dt.float32
F32R = mybir.dt.float32r
F16 = mybir.dt.float16

_nc = None


def _build():
    nc = bacc.Bacc("TRN2", target_bir_lowering=False, debug=False, num_devices=NCORES)
    xt = nc.dram_tensor("xt", [C, T], F32, kind="ExternalInput").ap()
    wq = nc.dram_tensor("wq", [128, 8 * H], F32, kind="ExternalInput").ap()
    wkv = nc.dram_tensor("wkv", [128, 8 * 2 * H], F32, kind="ExternalInput").ap()
    masks = nc.dram_tensor("masks", [128, 8 * TB], F16, kind="ExternalInput").ap()
    out = nc.dram_tensor("out", [NQB * TB, H], F32, kind="ExternalOutput").ap()

    TSB = 4 * TB  # 2048

    with tile.TileContext(nc) as tc:
        pid = nc.partition_id(engines=[mybir.EngineType.PE])
        s = pid % 2
        with tc.tile_pool(name="persist", bufs=1) as persist, \
             tc.tile_pool(name="x32p", bufs=6) as x32p, \
             tc.tile_pool(name="x16p", bufs=16) as x16p, \
             tc.tile_pool(name="vtp", bufs=2) as vtp, \
             tc.tile_pool(name="otp", bufs=2) as otp, \
             tc.tile_pool(name="obp", bufs=3) as obp, \
             tc.tile_pool(name="rcp", bufs=2) as rcp, \
             tc.tile_pool(name="ptp", bufs=6) as ptp, \
             tc.tile_pool(name="pjp", bufs=2, space="PSUM") as pj_pool, \
             tc.tile_pool(name="pvp", bufs=1, space="PSUM") as pv_pool, \
             tc.tile_pool(name="psp", bufs=2, space="PSUM") as ps_pool, \
             tc.tile_pool(name="pop", bufs=2, space="PSUM") as po_pool:
            ident = persist.tile([128, 128], F32)
            make_identity(nc, ident)
            wq_sb32 = persist.tile([128, 8 * H], F32)
            wkv_sb32 = persist.tile([128, 8 * 2 * H], F32)
            nc.scalar.dma_start(out=wq_sb32, in_=wq)
            nc.scalar.dma_start(out=wkv_sb32, in_=wkv)
            wq_sb = persist.tile([128, 8 * H], F16)
            wkv_sb = persist.tile([128, 8 * 2 * H], F16)
            nc.vector.tensor_copy(wq_sb, wq_sb32)
            nc.vector.tensor_copy(wkv_sb, wkv_sb32)
            masks_sb = persist.tile([128, 8 * TB], F16)
            nc.scalar.dma_start(out=masks_sb, in_=masks)

            QT = persist.tile([64, T], F16)           # Q^T on partitions 0:64
            KT = persist.tile([64, T], F16)           # K^T on partitions 0:64
            V = persist.tile([128, NKVT, H + 1], F16)  # [128, 65] per kv tile
            # col 64 of each kv tile = 1.0 (row-sum column); ACT rounds to f16
            nc.scalar.activation(
                V[:, :, H],
                ident[:, 0:NKVT],
                mybir.ActivationFunctionType.Copy,
                scale=0.0,
                bias=1.0,
            )

            # pre-warm the PE clock while the first x DMAs are in flight
            for w in range(8):
                psum_warm = ps_pool.tile([128, TB], F32, name="psum_warm",
                                         tag="ps")
                nc.tensor.matmul(
                    psum_warm[:, 0:128], ident, ident, start=True, stop=True
                )

            qoffs = [s * TB + i * 2 * TB for i in range(NQB)]
            x16s = [None] * 8

            # ---- attention emission machinery (fused into the proj stream) ----
            st = {"psum_o": None, "next_kp": [0] * NQB, "done": [False] * NQB,
                  "po": [None] * NQB}

            def emit_pair(i, kp):
                nkv = 8 * i + 8
                if kp == 0:
                    st["po"][i] = po_pool.tile([H + 1, TB], F32, name="psum_o",
                                               tag="po")
                psum_o = st["po"][i]
                for h in range(2):
                    k = 2 * kp + h
                    psum_s = ps_pool.tile([128, TB], F32, name="psum_s", tag="ps")
                    nc.tensor.matmul(
                        psum_s,
                        KT[:, k * 128:(k + 1) * 128],
                        QT[:, bass.ds(qoffs[i], TB)],
                        start=True,
                        stop=True,
                    )
                    pt = ptp.tile([128, TB], F16, name="pt", tag="pt")
                    nc.scalar.activation(
                        pt, psum_s, mybir.ActivationFunctionType.Exp, scale=0.125
                    )
                    j = k - (nkv - 8)
                    if j >= 0:
                        nc.vector.tensor_mul(
                            pt, pt, masks_sb[:, j * TB:(j + 1) * TB]
                        )
                    nc.tensor.matmul(
                        psum_o,
                        V[:, k, :],
                        pt,
                        start=(k == 0),
                        stop=(k == nkv - 1),
                    )
                if 2 * kp + 1 == nkv - 1:
                    # epilogue: normalize + store this q block
                    ot = otp.tile([H + 1, TB], F32)
                    nc.vector.tensor_copy(ot, psum_o)
                    for j2 in range(4):
                        psum_t = ps_pool.tile([128, TB], F32, name="psum_t",
                                              tag="ps")
                        nc.tensor.transpose(
                            psum_t[:, 0:H + 1],
                            ot[:, j2 * 128:(j2 + 1) * 128],
                            ident[0:H + 1, 0:H + 1],
                        )
                        rec = rcp.tile([128, 1], F32)
                        nc.vector.reciprocal(rec, psum_t[:, H:H + 1])
                        ob = obp.tile([128, H], F32)
                        nc.vector.tensor_scalar_mul(ob, psum_t[:, 0:H], rec)
                        nc.sync.dma_start(
                            out=out[i * TB + j2 * 128:i * TB + (j2 + 1) * 128, :],
                            in_=ob,
                        )

            def avail_g(i, kp):
                # q block i needs QT global block 2i+s (<= 2i+1); kv pair kp
                # needs proj t-block (2kp+1)//4. Block 3 intentionally waits
                # for the end anyway (its q arrives with the last proj block).
                base = max(2 * i + 1, (2 * kp + 1) // 4)
                if i == 2:
                    base = max(base, 6)
                return base

            def emit_ready(g, budget):
                emitted = 1
                while budget != 0 and emitted:
                    emitted = 0
                    for i in range(NQB):
                        if budget == 0:
                            break
                        kp = st["next_kp"][i]
                        if kp < (8 * i + 8) // 2 and avail_g(i, kp) <= g:
                            emit_pair(i, kp)
                            st["next_kp"][i] = kp + 1
                            emitted = 1
                            budget -= 1

            # ---- fused projection + attention stream ----
            PIECE = 2 * TB  # 1024
            for g in range(NTB):
                if g % 2 == 0:
                    # one [128, 1024] piece per c-chunk covers t-blocks g, g+1
                    p0 = g * TB
                    for c in range(8):
                        x32 = x32p.tile([128, PIECE], F32, name="x32", tag="x32")
                        nc.sync.dma_start(
                            out=x32,
                            in_=xt[c * 128:(c + 1) * 128, p0:p0 + PIECE],
                        )
                        x16 = x16p.tile([128, PIECE], F16, name="x16", tag="x16")
                        nc.vector.tensor_copy(x16, x32)
                        x16s[c] = x16
                sl = slice((g % 2) * TB, (g % 2 + 1) * TB)
                psum_vk = pj_pool.tile([128, TB], F32, name="psum_vk", tag="pj")
                for c in range(8):
                    nc.tensor.matmul(
                        psum_vk,
                        wkv_sb[:, c * 128:(c + 1) * 128],
                        x16s[c][:, sl],
                        start=(c == 0),
                        stop=(c == 7),
                    )
                psum_q = pj_pool.tile([64, TB], F32, name="psum_q", tag="pj")
                for c in range(8):
                    nc.tensor.matmul(
                        psum_q,
                        wq_sb[:, c * H:(c + 1) * H],
                        x16s[c][:, sl],
                        start=(c == 0),
                        stop=(c == 7),
                    )
                nc.scalar.copy(QT[:, g * TB:(g + 1) * TB], psum_q)
                nc.scalar.copy(KT[:, g * TB:(g + 1) * TB], psum_vk[0:64, :])
                vt = vtp.tile([128, TB], F32)
                nc.scalar.copy(vt[64:128, :], psum_vk[64:128, :])
                for j in range(4):
                    psum_v = pv_pool.tile([128, H], F32)
                    nc.tensor.transpose(
                        psum_v,
                        vt[64:128, j * 128:(j + 1) * 128],
                        ident[64:128, 64:128],
                    )
                    nc.scalar.copy(V[:, 4 * g + j, 0:H], psum_v)
                # attention filler: a few ready pairs per proj block
                emit_ready(g, 7 if g < NTB - 1 else -1)

    nc.compile()
    return nc


def get_nc():
    global _nc
    if _nc is None:
        _nc = _build()
    return _nc


def make_inputs(x, Wq, Wk, Wv):
    """Build the 8 per-core input maps."""
    x = np.asarray(x, dtype=np.float32)

    def pack_w(wt):
        # [C, M] (= W.T) -> [128, 8*M]: partition p, free c*M+m = wt[c*128+p, m]
        M = wt.shape[1]
        return np.ascontiguousarray(
            wt.reshape(8, 128, M).transpose(1, 0, 2).reshape(128, 8 * M)
        )

    wq_in = pack_w(np.asarray(Wq, np.float32).T)
    wkv_in = pack_w(
        np.concatenate(
            [np.asarray(Wk, np.float32).T, np.asarray(Wv, np.float32).T], axis=1
        )
    )
    p = np.arange(128, dtype=np.int64)[:, None]
    f = np.arange(TB, dtype=np.int64)[None, :]
    masks_by_s = []
    for s in range(2):
        m = np.concatenate(
            [((512 * s + f - 128 * j - p) >= 0).astype(np.float16) for j in range(8)],
            axis=1,
        )
        masks_by_s.append(np.ascontiguousarray(m))
    in_maps = []
    for core in range(NCORES):
        b, s = core // 2, core % 2
        in_maps.append(
            {
                "xt": np.ascontiguousarray(x[b].T),
                "wq": wq_in,
                "wkv": wkv_in,
                "masks": masks_by_s[s],
            }
        )
    return in_maps


def gather_output(results):
    """results: list of per-core {"out": [2048, 64]} -> full [B, T, H]."""
    O = np.empty((B, T, H), np.float32)
    for core in range(NCORES):
        b, s = core // 2, core % 2
        o = results[core]["out"]
        for i in range(NQB):
            g = 2 * i + s
            O[b, g * TB:(g + 1) * TB] = o[i * TB:(i + 1) * TB]
    return O


def kernel(x, Wq, Wk, Wv):
    nc = get_nc()
    in_maps = make_inputs(x, Wq, Wk, Wv)
    res = run_bass_kernel_spmd(nc, in_maps, list(range(NCORES)))
    return gather_output(res.results)
